# revision 1
# baseline (speedup 1.0000x reference)
"""Trainium2 Bass kernel for a causal-EMA encoder:

    out = EMA3(x @ W_down^T) @ W_up^T

with EMA layer i:  y_t = a_i * y_{t-1} + (1 - a_i) * h_t,  a_i = sigmoid(log_a[i]).

Shapes (hardcoded): x [4, 4096, 2048], W_down [512, 2048], W_up [2048, 512],
log_a [3, 512]. Output [4, 4096, 2048] fp32.

Strategy (8 NeuronCores, SPMD, no collectives):
  * Shard (batch, sequence-half): core c handles batch c//2, L-half c%2.
  * The EMA scans are causal with decay a ~ sigmoid(3) ≈ 0.95, so state
    contributions die off geometrically. Second-half cores recompute a
    KWARM-token "warmup" prefix instead of communicating carry state; the
    first-half cores get a zero-padded warmup so all cores run one program.
  * Linearity: scan_i((1-a_i) v) = (1-a_i) scan_i(v), so the three input
    injections fold into ONE per-channel pre-scale prod_i(1-a_i), then three
    pure a-decay scans, which map 1:1 onto the DVE TensorTensorScan ISA op.
  * All matmuls are fp16 (same PE throughput as bf16, 3 more mantissa bits),
    accumulating fp32 in PSUM. Scan state/carry stays fp32.
  * Transpose-free device code: the host feeds x already transposed per core
    as xT [D, LC] fp16 and receives outT [D, 2048] fp32, so the contraction
    dim is on partitions for every matmul and every DMA is wide-contiguous.
"""

import sys

for _p in ("/opt/trn_rl_repo", "/root/.axon_site/_ro/trn_rl_repo"):
    if _p not in sys.path:
        sys.path.append(_p)

import numpy as np
from contextlib import ExitStack

import concourse.tile as tile
from concourse import bacc, mybir
from concourse.bass_utils import run_bass_kernel_spmd

B, L, D, DI, NL = 4, 4096, 2048, 512, 3
P = 128
N_CORES = 8
HALF = L // 2          # tokens produced per core
CHUNK = 512            # l-chunk (= max fp32 PSUM free dim)
NKD = D // P           # 16 k-tiles for down-proj
NME = DI // P          # 4  e-tiles (down-proj m / up-proj k)
NMD = D // P           # 16 dd-tiles for up-proj

FP16 = mybir.dt.float16
F32 = mybir.dt.float32
MULT = mybir.AluOpType.mult
ADD = mybir.AluOpType.add

_module_cache: dict[int, object] = {}
LAST_RESULTS = None  # BassKernelResults of the most recent run (for profiling)


def _build_body(ctx: ExitStack, tc: tile.TileContext, kwarm: int):
    nc = tc.nc
    lc = HALF + kwarm
    # chunk widths: warmup chunks first (a single short chunk when
    # kwarm <= 512), then HALF//CHUNK full output chunks
    if kwarm <= CHUNK:
        warm_widths = [kwarm] if kwarm else []
    else:
        assert kwarm % CHUNK == 0
        warm_widths = [CHUNK] * (kwarm // CHUNK)
    widths = warm_widths + [CHUNK] * (HALF // CHUNK)
    warm_chunks = len(warm_widths)
    nchunk = len(widths)

    xT = nc.dram_tensor("xT", [D, lc], FP16, kind="ExternalInput").ap()
    wdT = nc.dram_tensor("wdT", [D, DI], FP16, kind="ExternalInput").ap()
    wuT = nc.dram_tensor("wuT", [DI, D], FP16, kind="ExternalInput").ap()
    # decay: a per (e-tile, channel, layer); scale: prod_i(1-a_i) per (e-tile, channel)
    decay = nc.dram_tensor("decay", [NME, P, NL], F32, kind="ExternalInput").ap()
    scale = nc.dram_tensor("scale", [NME, P, 1], F32, kind="ExternalInput").ap()
    outT = nc.dram_tensor("outT", [D, HALF], F32, kind="ExternalOutput").ap()

    singles = ctx.enter_context(tc.tile_pool(name="singles", bufs=1))
    xpool = ctx.enter_context(tc.tile_pool(name="xpool", bufs=3))
    hpool = ctx.enter_context(tc.tile_pool(name="hpool", bufs=4))
    zpool = ctx.enter_context(tc.tile_pool(name="zpool", bufs=4))
    zhpool = ctx.enter_context(tc.tile_pool(name="zhpool", bufs=8))
    opool = ctx.enter_context(tc.tile_pool(name="opool", bufs=8))
    psum_h = ctx.enter_context(tc.tile_pool(name="psum_h", bufs=2, space="PSUM"))
    psum_o = ctx.enter_context(tc.tile_pool(name="psum_o", bufs=6, space="PSUM"))

    # ---- persistent weights / per-channel constants ----
    # DMAs for these are emitted inside the chunk loop: down-proj weight
    # pieces interleave with the first x chunk so PE can start after ~1MB of
    # DMA instead of 6MB, and up-proj weights queue behind chunk 1's x.
    dec_sb = singles.tile([P, NME, NL], F32)
    sc_sb = singles.tile([P, NME, 1], F32)
    wd_sb = singles.tile([P, NKD, DI], FP16)
    wdTr = wdT.rearrange("(kt p) e -> p kt e", p=P)
    wu_sb = singles.tile([P, NME, D], FP16)

    # Per-(e-tile, layer) decay rows broadcast along the chunk (materialized
    # at j==0 below), since TensorTensorScan's data0 is a full [P, CHUNK]
    # tensor.
    ones = singles.tile([P, CHUNK], F32)
    nc.vector.memset(ones, 1.0)
    a_sb = singles.tile([P, NME, NL, CHUNK], F32)

    # Per-(e-tile, layer) scan carry state: last column of the previous
    # chunk's scan output. Separate tiny tiles so Tile's dependency tracking
    # serializes only the true per-(m, layer) carry chain.
    carry = [
        [
            singles.tile([P, 1], F32, tag=f"carry_{m}_{i}", name=f"carry_{m}_{i}")
            for i in range(NL)
        ]
        for m in range(NME)
    ]

    xTr = xT.rearrange("(kt p) l -> p kt l", p=P)
    outTr = outT.rearrange("(mt p) l -> p mt l", p=P)

    l0 = 0
    for j, w in enumerate(widths):
        x_sb = xpool.tile([P, NKD, CHUNK], FP16, tag="x")
        # k-tile DMA pieces so the k-loop can start on piece 0; on chunk 0
        # interleave the down-proj weight pieces with the x pieces, with
        # finer granularity up front so the first matmul starts sooner.
        pieces = [(0, 2), (2, 2), (4, 2), (6, 2), (8, 4), (12, 4)] if j == 0 else [
            (0, 4), (4, 4), (8, 4), (12, 4)
        ]
        for p0, sz in pieces:
            if j == 0:
                nc.sync.dma_start(
                    out=wd_sb[:, p0 : p0 + sz, :],
                    in_=wdTr[:, p0 : p0 + sz, :],
                )
            nc.sync.dma_start(
                out=x_sb[:, p0 : p0 + sz, :w],
                in_=xTr[:, p0 : p0 + sz, l0 : l0 + w],
            )
        if j == 0:
            # constants for the scans (needed ~6us in) load after the
            # critical path
            nc.sync.dma_start(out=dec_sb, in_=decay.rearrange("t p l -> p t l"))
            nc.sync.dma_start(out=sc_sb, in_=scale.rearrange("t p o -> p t o"))
            for t in range(NME):
                for i in range(NL):
                    nc.vector.tensor_scalar_mul(
                        a_sb[:, t, i, :], ones, dec_sb[:, t, i : i + 1]
                    )
        if j == min(1, nchunk - 1):
            # up-proj weights aren't needed until the first output chunk;
            # queue them behind chunk 1's x so that stream isn't delayed
            nc.sync.dma_start(out=wu_sb, in_=wuT.rearrange("(kt p) d -> p kt d", p=P))

        z3h = [None] * NME
        for m in range(NME):
            # ---- down-proj: h^T[e, l] = W_down^T.T @ x^T, contract over d ----
            ph = psum_h.tile([P, CHUNK], F32, tag="ph")
            for k in range(NKD):
                nc.tensor.matmul(
                    ph[:, :w],
                    lhsT=wd_sb[:, k, m * P : (m + 1) * P],
                    rhs=x_sb[:, k, :w],
                    start=(k == 0),
                    stop=(k == NKD - 1),
                )
            # evacuate PSUM (on ScalarE, keeping DVE free for the scans) with
            # the fused prod(1-a_i) input-injection scale
            hsc = hpool.tile([P, CHUNK], F32, tag="hsc")
            nc.scalar.mul(hsc[:, :w], ph[:, :w], sc_sb[:, m, 0:1])

            # ---- three chained EMA scans along the free (L) dim ----
            zin = hsc
            zlast = None
            for i in range(NL):
                zi = zpool.tile([P, CHUNK], F32, tag=f"z{i}")
                nc.vector.tensor_tensor_scan(
                    zi[:, :w], a_sb[:, m, i, :w], zin[:, :w],
                    initial=(0.0 if j == 0 else carry[m][i]),
                    op0=MULT, op1=ADD,
                )
                if j < nchunk - 1:
                    nc.vector.tensor_copy(out=carry[m][i], in_=zi[:, w - 1 : w])
                zin = zi
                zlast = zi

            if j >= warm_chunks:
                zh = zhpool.tile([P, CHUNK], FP16, tag="zh")
                nc.vector.tensor_copy(out=zh[:, :w], in_=zlast[:, :w])
                z3h[m] = zh

        if j >= warm_chunks:
            lo = l0 - kwarm
            # ---- up-proj: out^T[dd, l] = W_up^T.T @ y^T, contract over e ----
            for mm in range(NMD):
                po = psum_o.tile([P, CHUNK], F32, tag="po")
                for k in range(NME):
                    nc.tensor.matmul(
                        po[:, :w],
                        lhsT=wu_sb[:, k, mm * P : (mm + 1) * P],
                        rhs=z3h[k][:, :w],
                        start=(k == 0),
                        stop=(k == NME - 1),
                    )
                osb = opool.tile([P, CHUNK], F32, tag="osb")
                # alternate evacuations across ScalarE and DVE so neither
                # engine's queue paces the store stream or the kernel tail
                if mm % 2 == 1:
                    nc.vector.tensor_copy(out=osb[:, :w], in_=po[:, :w])
                else:
                    nc.scalar.copy(out=osb[:, :w], in_=po[:, :w])
                nc.sync.dma_start(out=outTr[:, mm, lo : lo + w], in_=osb[:, :w])
        l0 += w


def _get_module(kwarm: int):
    if kwarm in _module_cache:
        return _module_cache[kwarm]
    nc = bacc.Bacc("TRN2", target_bir_lowering=False, debug=False, enable_asserts=False)
    with tile.TileContext(nc) as tc:
        with ExitStack() as ctx:
            _build_body(ctx, tc, kwarm)
    nc.compile()
    _module_cache[kwarm] = nc
    return nc


def _pick_kwarm(a: np.ndarray) -> int:
    """Smallest KWARM (multiple of 64, capped) such that truncating scan
    history to KWARM tokens perturbs outputs by ~1e-5 of the h scale (an
    order below the fp16 matmul noise floor). 3-layer composed impulse
    response: the lag-k weight is (1-a)^3 * C(k+2,2) * a^k."""
    a64 = a.astype(np.float64)

    def tail(k):
        return float(np.max(0.5 * (k + 2) * (k + 1) * (a64**k) * (1.0 - a64) ** 3))

    k = 128
    while k < 2048 and tail(k) >= 2e-5:
        k += 64 if k < CHUNK else CHUNK
    return k


def kernel(x, W_down, W_up, log_a):
    global LAST_RESULTS
    x = np.ascontiguousarray(np.asarray(x, dtype=np.float32))
    W_down = np.asarray(W_down, dtype=np.float32)
    W_up = np.asarray(W_up, dtype=np.float32)
    log_a = np.asarray(log_a, dtype=np.float32)
    assert x.shape == (B, L, D) and W_down.shape == (DI, D) and W_up.shape == (D, DI)

    a64 = 1.0 / (1.0 + np.exp(-log_a.astype(np.float64)))          # [NL, DI]
    a = a64.astype(np.float32)
    scale = np.prod(1.0 - a64, axis=0).astype(np.float32)          # [DI]

    kwarm = _pick_kwarm(a)
    lc = HALF + kwarm
    nc = _get_module(kwarm)

    wdT = np.ascontiguousarray(W_down.T).astype(np.float16)
    wuT = np.ascontiguousarray(W_up.T).astype(np.float16)
    decay = np.ascontiguousarray(a.T.reshape(NME, P, NL))          # [t, p, l]
    scale_r = np.ascontiguousarray(scale.reshape(NME, P, 1))

    in_maps = []
    for c in range(N_CORES):
        b, h = divmod(c, 2)
        xt = np.zeros((lc, D), dtype=np.float32)
        lstart = h * HALF - kwarm
        src_lo = max(0, lstart)
        xt[src_lo - lstart :, :] = x[b, src_lo : h * HALF + HALF, :]
        xT = np.ascontiguousarray(xt.T).astype(np.float16)          # [D, lc]
        in_maps.append(
            {"xT": xT, "wdT": wdT, "wuT": wuT, "decay": decay, "scale": scale_r}
        )

    res = run_bass_kernel_spmd(nc, in_maps, core_ids=list(range(N_CORES)))
    LAST_RESULTS = res

    out = np.empty((B, L, D), dtype=np.float32)
    for c in range(N_CORES):
        b, h = divmod(c, 2)
        out[b, h * HALF : (h + 1) * HALF, :] = res.results[c]["outT"].T
    return out



# revision 8
# speedup vs baseline: 1.0733x; 1.0733x over previous
"""Trainium2 Bass kernel for a causal-EMA encoder:

    out = EMA3(x @ W_down^T) @ W_up^T

with EMA layer i:  y_t = a_i * y_{t-1} + (1 - a_i) * h_t,  a_i = sigmoid(log_a[i]).

Shapes (hardcoded): x [4, 4096, 2048], W_down [512, 2048], W_up [2048, 512],
log_a [3, 512]. Output [4, 4096, 2048] fp32.

Strategy (8 NeuronCores, SPMD, no collectives):
  * Shard (batch, sequence-half): core c handles batch c//2, L-half c%2.
    Second-half cores recompute a KWARM-token warmup prefix instead of
    communicating scan state (decay a ~ 0.95 makes history die geometrically).
  * All matmuls run in fp8e4 with MatmulPerfMode.DoubleRow (2 contraction
    slots per instruction at 0.5 PE cycles per output row). Each operand is
    split into hi + lo fp8 planes (hi = fp8(v), lo = fp8(v - hi)), and each
    product keeps the three big terms hi*hi + hi*lo + lo*hi:
      - down-proj: per k-pair one hi*hi instr; per k-tile one mixed instr
        computing w_lo^T x_hi + w_hi^T x_lo in its two slots.
      - up-proj: per k-tile one instr with duplicated-hi weights computing
        wu_hi^T (y_hi + y_lo); per k-pair one wu_lo^T y_hi instr.
    This is 0.75x the PE time of an fp16 kernel with ~0.25% end-to-end error.
  * fp8 needs power-of-2 pre-scaling so the lo planes stay out of subnormals:
    W* x64, x x8, y x8. The inverses fold into the existing per-channel PSUM
    evacuation scale (prod(1-a)/64) and a free host-side 2^-9 on the output.
  * The three EMA input injections fold into one pre-scale prod_i(1-a_i)
    (linearity), so the scans are pure a-decay TensorTensorScan ops on DVE.
  * Output is stored fp16 (values are 512*out, well within range; host
    converts and rescales), halving the store traffic.
"""

import sys

for _p in ("/opt/trn_rl_repo", "/root/.axon_site/_ro/trn_rl_repo"):
    if _p not in sys.path:
        sys.path.append(_p)

import numpy as np
import ml_dtypes
from contextlib import ExitStack

import concourse.tile as tile
from concourse import bacc, mybir
from concourse.bass_utils import run_bass_kernel_spmd

B, L, D, DI, NL = 4, 4096, 2048, 512, 3
P = 128
N_CORES = 8
HALF = L // 2          # tokens produced per core
KWARM = 256            # recomputed warmup tokens on second-half cores
LC = HALF + KWARM
CHUNK = 512            # l-chunk (= max fp32 PSUM free dim)
NKD = D // P           # 16 k-tiles for down-proj
NME = DI // P          # 4  e-tiles (down-proj m / up-proj k)
NMD = D // P           # 16 d-tiles for up-proj

FP16 = mybir.dt.float16
FP8 = mybir.dt.float8e4
F32 = mybir.dt.float32
MULT = mybir.AluOpType.mult
ADD = mybir.AluOpType.add
SUB = mybir.AluOpType.subtract
DR = mybir.MatmulPerfMode.DoubleRow

F8NP = ml_dtypes.float8_e4m3

WIDTHS = [KWARM] + [CHUNK] * (HALF // CHUNK)   # [256, 512, 512, 512, 512]
WARM_CHUNKS = 1

_module_cache: dict[str, object] = {}
LAST_RESULTS = None  # BassKernelResults of the most recent run (for profiling)


def _build_body(ctx: ExitStack, tc: tile.TileContext):
    nc = tc.nc
    nchunk = len(WIDTHS)

    nch = len(WIDTHS)
    # x is laid out per-chunk (warm chunk zero-padded to CHUNK) so every DMA
    # slice keeps hl+l contiguous (1024B runs, <=3 AP dims after balancing)
    x8 = nc.dram_tensor("x8", [nch, D, 2, CHUNK], FP8, kind="ExternalInput").ap()
    wd8 = nc.dram_tensor("wd8", [D, 2, DI], FP8, kind="ExternalInput").ap()
    wuhh = nc.dram_tensor("wuhh", [DI, 2, D], FP8, kind="ExternalInput").ap()
    wulo = nc.dram_tensor("wulo", [DI, D], FP8, kind="ExternalInput").ap()
    dec = nc.dram_tensor("dec", [NME, P, NL], F32, kind="ExternalInput").ap()
    sc = nc.dram_tensor("sc", [NME, P, 1], F32, kind="ExternalInput").ap()
    outT = nc.dram_tensor("outT", [D, HALF], FP16, kind="ExternalOutput").ap()

    singles = ctx.enter_context(tc.tile_pool(name="singles", bufs=1))
    xpool = ctx.enter_context(tc.tile_pool(name="xpool", bufs=2))
    opool = ctx.enter_context(tc.tile_pool(name="opool", bufs=6))
    psum_h = ctx.enter_context(tc.tile_pool(name="psum_h", bufs=2, space="PSUM"))
    psum_o = ctx.enter_context(tc.tile_pool(name="psum_o", bufs=6, space="PSUM"))

    # ---- persistent weights / constants ----
    wd_sb = singles.tile([P, NKD, 2, DI], FP8)     # hl = (lo, hi)
    wuhh_sb = singles.tile([P, NME, 2, D], FP8)    # hl = (hi, hi)
    wulo_sb = singles.tile([P, NME, D], FP8)
    dec_sb = singles.tile([P, NME, NL], F32)
    sc_sb = singles.tile([P, NME, 1], F32)

    # per-(e-tile, layer) decay rows broadcast along the chunk
    ones = singles.tile([P, CHUNK], F32)
    a_sb = singles.tile([P, NME, NL, CHUNK], F32)

    # scan chain tiles: explicit double-generation so chunk j+1's scan can
    # take its carry directly from chunk j's output tile (no carry copies)
    hsc_t = [
        [singles.tile([P, CHUNK], F32, name=f"hsc_{m}_{g}") for g in range(2)]
        for m in range(NME)
    ]
    z_t = [
        [
            [singles.tile([P, CHUNK], F32, name=f"z{i}_{m}_{g}") for g in range(2)]
            for m in range(NME)
        ]
        for i in range(NL)
    ]
    y8_t = [singles.tile([P, NME, 2, CHUNK], FP8, name=f"y8_{g}") for g in range(2)]

    x8r = x8.rearrange("c (kt p) hl l -> c p kt hl l", p=P)
    wd8r = wd8.rearrange("(kt p) hl e -> p kt hl e", p=P)
    wuhhr = wuhh.rearrange("(kt p) hl d -> p kt hl d", p=P)
    wulor = wulo.rearrange("(kt p) d -> p kt d", p=P)
    outTr = outT.rearrange("(mt p) l -> p mt l", p=P)

    # constants first: tiny DMAs, then the decay broadcasts on DVE while the
    # big weight/x DMAs stream
    nc.sync.dma_start(out=dec_sb, in_=dec.rearrange("t p l -> p t l"))
    nc.sync.dma_start(out=sc_sb, in_=sc.rearrange("t p o -> p t o"))
    nc.vector.memset(ones, 1.0)
    for t in range(NME):
        for i in range(NL):
            nc.vector.tensor_scalar_mul(
                a_sb[:, t, i, :], ones, dec_sb[:, t, i : i + 1]
            )

    l0s = np.cumsum([0] + WIDTHS).tolist()

    def emit_down(j: int):
        w = WIDTHS[j]
        g = j % 2
        x_sb = xpool.tile([P, NKD, 2, CHUNK], FP8, tag="x")
        # x DMA pieces, finer granularity up front; on chunk 0 interleave the
        # down-proj weight pieces, on chunk 1 the up-proj weights
        pieces = [(0, 2), (2, 2), (4, 4), (8, 4), (12, 4)]
        for pi, (p0, szk) in enumerate(pieces):
            if j == 0:
                nc.sync.dma_start(
                    out=wd_sb[:, p0 : p0 + szk], in_=wd8r[:, p0 : p0 + szk]
                )
            nc.sync.dma_start(
                out=x_sb[:, p0 : p0 + szk],
                in_=x8r[j, :, p0 : p0 + szk],
            )
            if j == 1 and pi in (1, 2, 3, 4):
                if pi <= 2:
                    h0 = (pi - 1) * 2
                    nc.sync.dma_start(
                        out=wuhh_sb[:, h0 : h0 + 2], in_=wuhhr[:, h0 : h0 + 2]
                    )
                elif pi == 3:
                    nc.sync.dma_start(out=wulo_sb, in_=wulor)

        for m in range(NME):
            ms = m * P
            ph = psum_h.tile([P, CHUNK], F32, tag="ph")
            first = True
            for kp in range(NKD // 2):
                ks = slice(2 * kp, 2 * kp + 2)
                nc.tensor.matmul(
                    ph[:, :w],
                    lhsT=wd_sb[:, ks, 1, ms : ms + P],
                    rhs=x_sb[:, ks, 0, :w],
                    start=first,
                    stop=False,
                    perf_mode=DR,
                )
                first = False
            for k in range(NKD):
                nc.tensor.matmul(
                    ph[:, :w],
                    lhsT=wd_sb[:, k, :, ms : ms + P],
                    rhs=x_sb[:, k, :, :w],
                    start=False,
                    stop=(k == NKD - 1),
                    perf_mode=DR,
                )
            # evacuate PSUM on ScalarE with the fused injection scale
            hsc = hsc_t[m][g]
            nc.scalar.mul(hsc[:, :w], ph[:, :w], sc_sb[:, m, 0:1])

            # three chained EMA scans along the free (L) dim
            zin = hsc
            for i in range(NL):
                zt = z_t[i][m][g]
                if j == 0:
                    init = 0.0
                else:
                    wprev = WIDTHS[j - 1]
                    init = z_t[i][m][1 - g][:, wprev - 1 : wprev]
                nc.vector.tensor_tensor_scan(
                    zt[:, :w], a_sb[:, m, i, :w], zin[:, :w],
                    initial=init, op0=MULT, op1=ADD,
                )
                zin = zt

            if j >= WARM_CHUNKS:
                # split scan output into hi/lo fp8 planes for the up-proj
                # (on GpSimd: SBUF-only ops, keeps DVE free for the scans)
                y8 = y8_t[g]
                nc.gpsimd.tensor_copy(out=y8[:, m, 0, :w], in_=zin[:, :w])
                nc.gpsimd.tensor_tensor(
                    out=y8[:, m, 1, :w], in0=zin[:, :w], in1=y8[:, m, 0, :w],
                    op=SUB,
                )

    def emit_up(j: int):
        w = WIDTHS[j]
        lo = l0s[j] - KWARM
        y8 = y8_t[j % 2]
        for mm in range(NMD):
            mms = mm * P
            po = psum_o.tile([P, CHUNK], F32, tag="po")
            first = True
            for k in range(NME):
                nc.tensor.matmul(
                    po[:, :w],
                    lhsT=wuhh_sb[:, k, :, mms : mms + P],
                    rhs=y8[:, k, :, :w],
                    start=first,
                    stop=False,
                    perf_mode=DR,
                )
                first = False
            for kp in range(NME // 2):
                ks = slice(2 * kp, 2 * kp + 2)
                nc.tensor.matmul(
                    po[:, :w],
                    lhsT=wulo_sb[:, ks, mms : mms + P],
                    rhs=y8[:, ks, 0, :w],
                    start=False,
                    stop=(kp == NME // 2 - 1),
                    perf_mode=DR,
                )
            osb = opool.tile([P, CHUNK], FP16, tag="osb")
            # GpSimd cannot read PSUM: alternate evacuations ScalarE/DVE
            if mm % 4 == 3:
                nc.vector.tensor_copy(out=osb[:, :w], in_=po[:, :w])
            else:
                nc.scalar.copy(out=osb[:, :w], in_=po[:, :w])
            nc.sync.dma_start(out=outTr[:, mm, lo : lo + w], in_=osb[:, :w])

    for j in range(nchunk):
        emit_down(j)
        if j - 1 >= WARM_CHUNKS:
            emit_up(j - 1)
    emit_up(nchunk - 1)


def _get_module():
    if "m" in _module_cache:
        return _module_cache["m"]
    nc = bacc.Bacc("TRN2", target_bir_lowering=False, debug=False, enable_asserts=False)
    with tile.TileContext(nc) as tc:
        with ExitStack() as ctx:
            _build_body(ctx, tc)
    nc.compile()
    _module_cache["m"] = nc
    return nc


def _split8(v: np.ndarray, scale: float):
    """hi/lo fp8 planes of v*scale (pow2 scale keeps lo out of subnormals)."""
    vs = (v * scale).astype(np.float32)
    hi = vs.astype(F8NP)
    lo = (vs - hi.astype(np.float32)).astype(F8NP)
    return hi, lo


def kernel(x, W_down, W_up, log_a):
    global LAST_RESULTS
    x = np.ascontiguousarray(np.asarray(x, dtype=np.float32))
    W_down = np.asarray(W_down, dtype=np.float32)
    W_up = np.asarray(W_up, dtype=np.float32)
    log_a = np.asarray(log_a, dtype=np.float32)
    assert x.shape == (B, L, D) and W_down.shape == (DI, D) and W_up.shape == (D, DI)

    a64 = 1.0 / (1.0 + np.exp(-log_a.astype(np.float64)))          # [NL, DI]
    a = a64.astype(np.float32)
    scale = np.prod(1.0 - a64, axis=0)                             # [DI]

    nc = _get_module()

    wdh, wdl = _split8(np.ascontiguousarray(W_down.T), 64.0)       # [D, DI]
    wd8 = np.ascontiguousarray(np.stack([wdl, wdh], axis=1))       # (lo, hi)
    wuh, wul = _split8(np.ascontiguousarray(W_up.T), 64.0)         # [DI, D]
    wuhh_a = np.ascontiguousarray(np.stack([wuh, wuh], axis=1))
    wulo_a = np.ascontiguousarray(wul)
    dec_a = np.ascontiguousarray(a.T.reshape(NME, P, NL))
    sc_a = np.ascontiguousarray(
        (scale / 64.0).astype(np.float32).reshape(NME, P, 1)
    )

    nch = len(WIDTHS)
    l0s = np.cumsum([0] + WIDTHS).tolist()
    in_maps = []
    for c in range(N_CORES):
        b, h = divmod(c, 2)
        xt = np.zeros((LC, D), dtype=np.float32)
        lstart = h * HALF - KWARM
        src_lo = max(0, lstart)
        xt[src_lo - lstart :, :] = x[b, src_lo : h * HALF + HALF, :]
        xh, xl = _split8(xt.T, 8.0)                                # [D, LC]
        x8_a = np.zeros((nch, D, 2, CHUNK), dtype=F8NP)            # (hi, lo)
        for j in range(nch):
            w = WIDTHS[j]
            x8_a[j, :, 0, :w] = xh[:, l0s[j] : l0s[j] + w]
            x8_a[j, :, 1, :w] = xl[:, l0s[j] : l0s[j] + w]
        in_maps.append(
            {
                "x8": x8_a, "wd8": wd8, "wuhh": wuhh_a, "wulo": wulo_a,
                "dec": dec_a, "sc": sc_a,
            }
        )

    res = run_bass_kernel_spmd(nc, in_maps, core_ids=list(range(N_CORES)))
    LAST_RESULTS = res

    out = np.empty((B, L, D), dtype=np.float32)
    for c in range(N_CORES):
        b, h = divmod(c, 2)
        o = res.results[c]["outT"].astype(np.float32) * (2.0 ** -9)
        out[b, h * HALF : (h + 1) * HALF, :] = o.T
    return out


# revision 12
# speedup vs baseline: 1.1390x; 1.0612x over previous
"""Trainium2 Bass kernel for a causal-EMA encoder:

    out = EMA3(x @ W_down^T) @ W_up^T

with EMA layer i:  y_t = a_i * y_{t-1} + (1 - a_i) * h_t,  a_i = sigmoid(log_a[i]).

Shapes (hardcoded): x [4, 4096, 2048], W_down [512, 2048], W_up [2048, 512],
log_a [3, 512]. Output [4, 4096, 2048] fp32.

Strategy (8 NeuronCores, SPMD, no collectives):
  * Shard (batch, sequence-half): core c handles batch c//2, L-half c%2.
    Second-half cores recompute a KWARM-token warmup prefix instead of
    communicating scan state (decay a ~ 0.95 makes history die geometrically).
  * All matmuls run in fp8e4 with MatmulPerfMode.DoubleRow (2 contraction
    slots per instruction at 0.5 PE cycles per output row). Each operand is
    split into hi + lo fp8 planes (hi = fp8(v), lo = fp8(v - hi)), and each
    product keeps the three big terms hi*hi + hi*lo + lo*hi:
      - down-proj: per k-pair one hi*hi instr; per k-tile one mixed instr
        computing w_lo^T x_hi + w_hi^T x_lo in its two slots.
      - up-proj: per k-tile one instr with duplicated-hi weights computing
        wu_hi^T (y_hi + y_lo); per k-pair one wu_lo^T y_hi instr.
    This is 0.75x the PE time of an fp16 kernel with ~0.25% end-to-end error.
  * fp8 needs power-of-2 pre-scaling so the lo planes stay out of subnormals:
    W* x64, x x8, y x8. The inverses fold into the existing per-channel PSUM
    evacuation scale (prod(1-a)/64) and a free host-side 2^-9 on the output.
  * The three EMA input injections fold into one pre-scale prod_i(1-a_i)
    (linearity), so the scans are pure a-decay TensorTensorScan ops on DVE.
  * Output is stored fp16 (values are 512*out, well within range; host
    converts and rescales), halving the store traffic.
  * Schedule: x arrives per-chunk (exact-width DRAM tensors so every DMA is
    >=512B-contiguous); early chunks run the down-proj k-outer so the PE
    consumes DMA pieces as they land; the up-proj trails the down-proj by one
    chunk; the sequence ends with 384/128-token chunks to shrink the tail.
"""

import sys

for _p in ("/opt/trn_rl_repo", "/root/.axon_site/_ro/trn_rl_repo"):
    if _p not in sys.path:
        sys.path.append(_p)

import numpy as np
import ml_dtypes
from contextlib import ExitStack

import concourse.tile as tile
from concourse import bacc, mybir
from concourse.bass_utils import run_bass_kernel_spmd

B, L, D, DI, NL = 4, 4096, 2048, 512, 3
P = 128
N_CORES = 8
HALF = L // 2          # tokens produced per core
KWARM = 256            # recomputed warmup tokens on second-half cores
LC = HALF + KWARM
CHUNK = 512            # max l-chunk (= fp32 PSUM bank free dim)
NKD = D // P           # 16 k-tiles for down-proj
NME = DI // P          # 4  e-tiles (down-proj m / up-proj k)
NMD = D // P           # 16 d-tiles for up-proj

FP16 = mybir.dt.float16
FP8 = mybir.dt.float8e4
F32 = mybir.dt.float32
MULT = mybir.AluOpType.mult
ADD = mybir.AluOpType.add
SUB = mybir.AluOpType.subtract
DR = mybir.MatmulPerfMode.DoubleRow

F8NP = ml_dtypes.float8_e4m3

WIDTHS = [KWARM, 512, 512, 512, 384, 128]   # warm + 2048 output tokens
WARM_CHUNKS = 1
KOUTER_CHUNKS = 3      # chunks emitted k-outer (consume x pieces as they land)
NCH = len(WIDTHS)
L0S = [0]
for _w in WIDTHS:
    L0S.append(L0S[-1] + _w)

_module_cache: dict[str, object] = {}
LAST_RESULTS = None  # BassKernelResults of the most recent run (for profiling)


def _build_body(ctx: ExitStack, tc: tile.TileContext):
    nc = tc.nc

    xds = [
        nc.dram_tensor(f"x8_{j}", [D, 2, WIDTHS[j]], FP8, kind="ExternalInput")
        .ap()
        .rearrange("(kt p) hl l -> p kt hl l", p=P)
        for j in range(NCH)
    ]
    wd8 = nc.dram_tensor("wd8", [D, 2, DI], FP8, kind="ExternalInput").ap()
    wuhh = nc.dram_tensor("wuhh", [DI, 2, D], FP8, kind="ExternalInput").ap()
    wulo = nc.dram_tensor("wulo", [DI, D], FP8, kind="ExternalInput").ap()
    dec = nc.dram_tensor("dec", [NME, P, NL], F32, kind="ExternalInput").ap()
    sc = nc.dram_tensor("sc", [NME, P, 1], F32, kind="ExternalInput").ap()
    outT = nc.dram_tensor("outT", [D, HALF], FP16, kind="ExternalOutput").ap()

    singles = ctx.enter_context(tc.tile_pool(name="singles", bufs=1))
    xpool = ctx.enter_context(tc.tile_pool(name="xpool", bufs=3))
    opool = ctx.enter_context(tc.tile_pool(name="opool", bufs=6))
    psum_h = ctx.enter_context(tc.tile_pool(name="psum_h", bufs=4, space="PSUM"))
    psum_o = ctx.enter_context(tc.tile_pool(name="psum_o", bufs=4, space="PSUM"))

    # ---- persistent weights / constants ----
    wd_sb = singles.tile([P, NKD, 2, DI], FP8)     # hl = (lo, hi)
    wuhh_sb = singles.tile([P, NME, 2, D], FP8)    # hl = (hi, hi)
    wulo_sb = singles.tile([P, NME, D], FP8)
    dec_sb = singles.tile([P, NME, NL], F32)
    sc_sb = singles.tile([P, NME, 1], F32)

    # per-(e-tile, layer) decay rows broadcast along the chunk
    ones = singles.tile([P, CHUNK], F32)
    a_sb = singles.tile([P, NME, NL, CHUNK], F32)

    # scan chain tiles: explicit double-generation so chunk j+1's scan can
    # take its carry directly from chunk j's output tile (no carry copies)
    hsc_t = [
        [singles.tile([P, CHUNK], F32, name=f"hsc_{m}_{g}") for g in range(2)]
        for m in range(NME)
    ]
    z_t = [
        [
            [singles.tile([P, CHUNK], F32, name=f"z{i}_{m}_{g}") for g in range(2)]
            for m in range(NME)
        ]
        for i in range(NL)
    ]
    y8_t = [singles.tile([P, NME, 2, CHUNK], FP8, name=f"y8_{g}") for g in range(2)]

    wd8r = wd8.rearrange("(kt p) hl e -> p kt hl e", p=P)
    wuhhr = wuhh.rearrange("(kt p) hl d -> p kt hl d", p=P)
    wulor = wulo.rearrange("(kt p) d -> p kt d", p=P)
    outTr = outT.rearrange("(mt p) l -> p mt l", p=P)

    # constants first: tiny DMAs, then the decay broadcasts on DVE while the
    # big weight/x DMAs stream
    nc.sync.dma_start(out=dec_sb, in_=dec.rearrange("t p l -> p t l"))
    nc.sync.dma_start(out=sc_sb, in_=sc.rearrange("t p o -> p t o"))
    nc.vector.memset(ones, 1.0)
    for t in range(NME):
        for i in range(NL):
            nc.vector.tensor_scalar_mul(
                a_sb[:, t, i, :], ones, dec_sb[:, t, i : i + 1]
            )

    x_sbs = {}

    def emit_xdma(j: int, interleave=()):
        """DMA chunk j's x in k-tile pieces, optionally interleaving other
        (weight) DMA thunks between pieces."""
        w = WIDTHS[j]
        x_sb = xpool.tile([P, NKD, 2, w], FP8, tag="x", name=f"x_sb_{j}")
        x_sbs[j] = x_sb
        others = list(interleave)
        for pi, (p0, szk) in enumerate([(0, 2), (2, 2), (4, 4), (8, 4), (12, 4)]):
            if pi < len(others):
                others[pi]()
            nc.sync.dma_start(
                out=x_sb[:, p0 : p0 + szk], in_=xds[j][:, p0 : p0 + szk]
            )
        for o in others[5:]:
            o()

    def down_matmul(ph, x_sb, m, kp_or_k, cross, w, first, last):
        ms = m * P
        if not cross:
            ks = slice(2 * kp_or_k, 2 * kp_or_k + 2)
            nc.tensor.matmul(
                ph[:, :w],
                lhsT=wd_sb[:, ks, 1, ms : ms + P],
                rhs=x_sb[:, ks, 0, :w],
                start=first, stop=last, perf_mode=DR,
            )
        else:
            k = kp_or_k
            nc.tensor.matmul(
                ph[:, :w],
                lhsT=wd_sb[:, k, :, ms : ms + P],
                rhs=x_sb[:, k, :, :w],
                start=first, stop=last, perf_mode=DR,
            )

    def emit_down_tail(j, m, ph):
        """PSUM evacuate + scans + y8 split for one m-tile of chunk j."""
        w = WIDTHS[j]
        g = j % 2
        hsc = hsc_t[m][g]
        nc.scalar.mul(hsc[:, :w], ph[:, :w], sc_sb[:, m, 0:1])
        zin = hsc
        for i in range(NL):
            zt = z_t[i][m][g]
            if j == 0:
                init = 0.0
            else:
                wprev = WIDTHS[j - 1]
                init = z_t[i][m][1 - g][:, wprev - 1 : wprev]
            nc.vector.tensor_tensor_scan(
                zt[:, :w], a_sb[:, m, i, :w], zin[:, :w],
                initial=init, op0=MULT, op1=ADD,
            )
            zin = zt
        if j >= WARM_CHUNKS:
            y8 = y8_t[g]
            nc.gpsimd.tensor_copy(out=y8[:, m, 0, :w], in_=zin[:, :w])
            nc.gpsimd.tensor_tensor(
                out=y8[:, m, 1, :w], in0=zin[:, :w], in1=y8[:, m, 0, :w], op=SUB,
            )

    def emit_down(j: int):
        w = WIDTHS[j]
        x_sb = x_sbs[j]
        phs = [
            psum_h.tile([P, CHUNK], F32, tag="ph", name=f"ph_{j}_{m}")
            for m in range(NME)
        ]
        if j < KOUTER_CHUNKS:
            # k-outer: consume x/wd DMA pieces as they land
            for kp in range(NKD // 2):
                for m in range(NME):
                    down_matmul(phs[m], x_sb, m, kp, False, w, kp == 0, False)
                for k in (2 * kp, 2 * kp + 1):
                    for m in range(NME):
                        down_matmul(
                            phs[m], x_sb, m, k, True, w, False, k == NKD - 1
                        )
            for m in range(NME):
                emit_down_tail(j, m, phs[m])
        else:
            for m in range(NME):
                for kp in range(NKD // 2):
                    down_matmul(phs[m], x_sb, m, kp, False, w, kp == 0, False)
                for k in range(NKD):
                    down_matmul(
                        phs[m], x_sb, m, k, True, w, False, k == NKD - 1
                    )
                emit_down_tail(j, m, phs[m])

    def emit_up(j: int):
        w = WIDTHS[j]
        lo = L0S[j] - KWARM
        y8 = y8_t[j % 2]
        for mm in range(NMD):
            mms = mm * P
            po = psum_o.tile([P, CHUNK], F32, tag="po")
            first = True
            for k in range(NME):
                nc.tensor.matmul(
                    po[:, :w],
                    lhsT=wuhh_sb[:, k, :, mms : mms + P],
                    rhs=y8[:, k, :, :w],
                    start=first, stop=False, perf_mode=DR,
                )
                first = False
            for kp in range(NME // 2):
                ks = slice(2 * kp, 2 * kp + 2)
                nc.tensor.matmul(
                    po[:, :w],
                    lhsT=wulo_sb[:, ks, mms : mms + P],
                    rhs=y8[:, ks, 0, :w],
                    start=False, stop=(kp == NME // 2 - 1), perf_mode=DR,
                )
            osb = opool.tile([P, CHUNK], FP16, tag="osb")
            # GpSimd cannot read PSUM: alternate evacuations ScalarE/DVE
            if mm % 4 == 3:
                nc.vector.tensor_copy(out=osb[:, :w], in_=po[:, :w])
            else:
                nc.scalar.copy(out=osb[:, :w], in_=po[:, :w])
            nc.sync.dma_start(out=outTr[:, mm, lo : lo + w], in_=osb[:, :w])

    # ---- schedule ----
    # DMA order: x0 (interleaved with wd), x1 | x2 (interleaved with wu) |
    # x(j+1) at iteration j. Up-proj trails down-proj by one chunk.
    wd_pieces = [
        (lambda p0=p0, szk=szk: nc.sync.dma_start(
            out=wd_sb[:, p0 : p0 + szk], in_=wd8r[:, p0 : p0 + szk]
        ))
        for p0, szk in [(0, 2), (2, 2), (4, 4), (8, 4), (12, 4)]
    ]
    wu_pieces = [
        lambda: nc.sync.dma_start(out=wuhh_sb[:, 0:2], in_=wuhhr[:, 0:2]),
        lambda: nc.sync.dma_start(out=wuhh_sb[:, 2:4], in_=wuhhr[:, 2:4]),
        lambda: nc.sync.dma_start(out=wulo_sb, in_=wulor),
    ]
    for j in range(NCH):
        if j == 0:
            emit_xdma(0, interleave=wd_pieces)
            emit_xdma(1)
        elif j == 1:
            emit_xdma(2, interleave=wu_pieces)
        elif j + 1 < NCH:
            emit_xdma(j + 1)
        emit_down(j)
        if j - 1 >= WARM_CHUNKS:
            emit_up(j - 1)
    emit_up(NCH - 1)


def _get_module():
    if "m" in _module_cache:
        return _module_cache["m"]
    nc = bacc.Bacc("TRN2", target_bir_lowering=False, debug=False, enable_asserts=False)
    with tile.TileContext(nc) as tc:
        with ExitStack() as ctx:
            _build_body(ctx, tc)
    nc.compile()
    _module_cache["m"] = nc
    return nc


def _split8(v: np.ndarray, scale: float):
    """hi/lo fp8 planes of v*scale (pow2 scale keeps lo out of subnormals)."""
    vs = (v * scale).astype(np.float32)
    hi = vs.astype(F8NP)
    lo = (vs - hi.astype(np.float32)).astype(F8NP)
    return hi, lo


def kernel(x, W_down, W_up, log_a):
    global LAST_RESULTS
    x = np.ascontiguousarray(np.asarray(x, dtype=np.float32))
    W_down = np.asarray(W_down, dtype=np.float32)
    W_up = np.asarray(W_up, dtype=np.float32)
    log_a = np.asarray(log_a, dtype=np.float32)
    assert x.shape == (B, L, D) and W_down.shape == (DI, D) and W_up.shape == (D, DI)

    a64 = 1.0 / (1.0 + np.exp(-log_a.astype(np.float64)))          # [NL, DI]
    a = a64.astype(np.float32)
    scale = np.prod(1.0 - a64, axis=0)                             # [DI]

    nc = _get_module()

    wdh, wdl = _split8(np.ascontiguousarray(W_down.T), 64.0)       # [D, DI]
    wd8 = np.ascontiguousarray(np.stack([wdl, wdh], axis=1))       # (lo, hi)
    wuh, wul = _split8(np.ascontiguousarray(W_up.T), 64.0)         # [DI, D]
    wuhh_a = np.ascontiguousarray(np.stack([wuh, wuh], axis=1))
    wulo_a = np.ascontiguousarray(wul)
    dec_a = np.ascontiguousarray(a.T.reshape(NME, P, NL))
    sc_a = np.ascontiguousarray(
        (scale / 64.0).astype(np.float32).reshape(NME, P, 1)
    )

    in_maps = []
    for c in range(N_CORES):
        b, h = divmod(c, 2)
        xt = np.zeros((LC, D), dtype=np.float32)
        lstart = h * HALF - KWARM
        src_lo = max(0, lstart)
        xt[src_lo - lstart :, :] = x[b, src_lo : h * HALF + HALF, :]
        xh, xl = _split8(xt.T, 8.0)                                # [D, LC]
        m = {
            "wd8": wd8, "wuhh": wuhh_a, "wulo": wulo_a,
            "dec": dec_a, "sc": sc_a,
        }
        for j in range(NCH):
            w = WIDTHS[j]
            xj = np.empty((D, 2, w), dtype=F8NP)
            xj[:, 0, :] = xh[:, L0S[j] : L0S[j] + w]
            xj[:, 1, :] = xl[:, L0S[j] : L0S[j] + w]
            m[f"x8_{j}"] = xj
        in_maps.append(m)

    res = run_bass_kernel_spmd(nc, in_maps, core_ids=list(range(N_CORES)))
    LAST_RESULTS = res

    out = np.empty((B, L, D), dtype=np.float32)
    for c in range(N_CORES):
        b, h = divmod(c, 2)
        o = res.results[c]["outT"].astype(np.float32) * (2.0 ** -9)
        out[b, h * HALF : (h + 1) * HALF, :] = o.T
    return out
